# revision 8
# baseline (speedup 1.0000x reference)
"""ButterflyBlock sparse-attention kernel for 8 Trainium2 NeuronCores, v3.

Full inputs in, full output out. 32 butterfly blocks data-parallel, 4 per
core. QKVO weights resident in SBUF. Per block: V projection chains, then 8
head-pair steps interleaving scores (quadrant ping-pong, both heads packed
into one 2-bank PSUM tile per m-chunk), exp on scalar, PV chains pipelined
one pair behind, and filler chains (prev block's Wo units + next block's
Q/K projections) so the pairs phase stays PE-bound while scalar runs exp.

Hardcoded shape: x [4, 4096, 1024], D=1024, H=16, dh=64, CHUNK=256, L=512.
"""

import sys

sys.path.insert(0, "/root/.axon_site/_ro/trn_rl_repo")
sys.path.insert(0, "/opt/trn_rl_repo")

import ml_dtypes
import numpy as np

import concourse.bass as bass
import concourse.bacc as bacc
import concourse.mybir as mybir
import concourse.tile as tile
from concourse.bass_utils import run_bass_kernel_spmd

F32 = mybir.dt.float32
BF16 = mybir.dt.bfloat16

B, N, D = 4, 4096, 1024
H, DH = 16, 64
CHUNK = 256
L = 2 * CHUNK          # 512 tokens per block
NBLK = 4               # blocks per core
NCORES = 8
KC = D // 128          # 8 contraction chunks
LC = L // 128          # 4 token chunks
NP = H // 2            # 8 head pairs
EXP_FUNC = mybir.ActivationFunctionType.Exp
IDENT = mybir.ActivationFunctionType.Identity

# v_sb free layout per m-chunk: 16 head-blocks of 128 cols each;
# even head: [v_h(64)|ones(64)], odd head: [ones(64)|v_h(64)]
VW = H * 128           # 2048


def _build_nc(has_bq, has_bk, has_bv):
    nc = bacc.Bacc("TRN2", target_bir_lowering=False, debug=False)

    zt = nc.dram_tensor("zt", [NBLK, D, L], BF16, kind="ExternalInput")
    wq = nc.dram_tensor("wq", [D, D], BF16, kind="ExternalInput")
    wk = nc.dram_tensor("wk", [D, D], BF16, kind="ExternalInput")
    wv = nc.dram_tensor("wv", [D, D], BF16, kind="ExternalInput")
    wo = nc.dram_tensor("wo", [D, D], BF16, kind="ExternalInput")
    ones = nc.dram_tensor("ones", [128, 64], BF16, kind="ExternalInput")
    y = nc.dram_tensor("y", [NBLK, L, D], BF16, kind="ExternalOutput")
    bq = bk = bv = None
    if has_bq:
        bq = nc.dram_tensor("bq", [128, KC], F32, kind="ExternalInput")
    if has_bk:
        bk = nc.dram_tensor("bk", [128, KC], F32, kind="ExternalInput")
    if has_bv:
        bv = nc.dram_tensor("bv", [128, KC], F32, kind="ExternalInput")

    with tile.TileContext(nc) as tc:
        with (
            tc.tile_pool(name="wpool", bufs=1) as wpool,
            tc.tile_pool(name="zpool", bufs=3) as zpool,
            tc.tile_pool(name="qkpool", bufs=4) as qkpool,
            tc.tile_pool(name="vpool", bufs=2) as vpool,
            tc.tile_pool(name="ppool", bufs=2) as ppool,
            tc.tile_pool(name="upool", bufs=2) as upool,
            tc.tile_pool(name="rpool", bufs=3) as rpool,
            tc.tile_pool(name="ysb", bufs=3) as ypool,
            tc.tile_pool(name="bias", bufs=1) as bpool,
            tc.tile_pool(name="mmps", bufs=4, space="PSUM") as mmps,
            tc.tile_pool(name="scps", bufs=2, space="PSUM") as scps,
        ):
            bq_sb = bk_sb = bv_sb = None
            if has_bq:
                bq_sb = bpool.tile([128, KC], F32)
                nc.sync.dma_start(bq_sb[:], bq[:])
            if has_bk:
                bk_sb = bpool.tile([128, KC], F32)
                nc.sync.dma_start(bk_sb[:], bk[:])
            if has_bv:
                bv_sb = bpool.tile([128, KC], F32)
                nc.sync.dma_start(bv_sb[:], bv[:])

            zt_tiles = [None] * NBLK

            def load_zt(blk, eng=None, engs=None):
                t = zpool.tile([128, KC, L], BF16)
                zt_r = zt[blk].rearrange("(kc p) l -> p kc l", p=128)
                if engs:
                    for kc in range(KC):
                        engs[kc].dma_start(t[:, kc, :], zt_r[:, kc, :])
                else:
                    (eng or nc.sync).dma_start(t[:], zt_r[:])
                zt_tiles[blk] = t

            # block-0 zt rows interleaved with wq rows in issue order:
            # the first Q chain's mm(kc) needs exactly (zt row kc, wq row
            # kc), so pairwise arrival paces the chain with minimal head.
            t0 = zpool.tile([128, KC, L], BF16, tag="t", name="t0")
            zt0_r = zt[0].rearrange("(kc p) l -> p kc l", p=128)
            zt_tiles[0] = t0
            wq_sb = wpool.tile([128, KC, D], BF16, tag="wq")
            wq_r = wq.rearrange("(kc p) d -> p kc d", p=128)
            for kc in range(KC):
                nc.sync.dma_start(t0[:, kc, :], zt0_r[:, kc, :])
                nc.sync.dma_start(wq_sb[:, kc, :], wq_r[:, kc, :])
            wk_sb = wpool.tile([128, KC, D], BF16, tag="wk")
            wk_r = wk.rearrange("(kc p) d -> p kc d", p=128)
            for kc in range(KC):
                nc.sync.dma_start(wk_sb[:, kc, :], wk_r[:, kc, :])
            load_zt(1)
            wv_sb = wpool.tile([128, KC, D], BF16, tag="wv")
            wv_r = wv.rearrange("(kc p) d -> p kc d", p=128)
            nc.sync.dma_start(wv_sb[:], wv_r[:])
            wo_sb = wpool.tile([128, KC, D], BF16, tag="wo")
            wo_r = wo.rearrange("(kc p) d -> p kc d", p=128)
            nc.sync.dma_start(wo_sb[:], wo_r[:])

            # ---- emission helpers ------------------------------------
            def proj_chain(w_sb, zt_sb, out_sb, dc, b_sb):
                """One projection dc-chunk: out_sb[:, dc, :] (w^T z layout)."""
                ps = mmps.tile([128, L], F32, tag="ps")
                for kc in range(KC):
                    nc.tensor.matmul(
                        ps[:],
                        w_sb[:, kc, dc * 128:(dc + 1) * 128].opt(),
                        zt_sb[:, kc, :].opt(),
                        start=(kc == 0),
                        stop=(kc == KC - 1),
                    )
                if b_sb is not None:
                    nc.scalar.activation(
                        out_sb[:, dc, :], ps[:], IDENT,
                        bias=b_sb[:, dc:dc + 1], scale=1.0,
                    )
                else:
                    nc.vector.tensor_copy(out_sb[:, dc, :], ps[:])

            def v_chain(blk, v_sb, lc, nh):
                """V projection chunk into ones-augmented layout."""
                zt_sb = zt_tiles[blk]
                ps = mmps.tile([128, L], F32, tag="ps")
                for kc in range(KC):
                    nc.tensor.matmul(
                        ps[:],
                        zt_sb[:, kc, lc * 128:(lc + 1) * 128].opt(),
                        wv_sb[:, kc, nh * 512:(nh + 1) * 512].opt(),
                        start=(kc == 0),
                        stop=(kc == KC - 1),
                    )
                base = v_sb[:, lc, :]
                for par in range(2):  # even / odd heads of this half
                    dst = bass.AP(
                        tensor=base.tensor,
                        offset=base.offset + (nh * 8 + par) * 128 + par * 64,
                        ap=[list(base.ap[0]), [256, 4], [1, 64]],
                    )
                    src = bass.AP(
                        tensor=ps.tensor,
                        offset=ps[:].offset + par * 64,
                        ap=[list(ps[:].ap[0]), [128, 4], [1, 64]],
                    )
                    nc.vector.tensor_copy(dst, src)

            def v_ones(v_sb):
                ones_b = bass.AP(
                    tensor=ones[:].tensor, offset=ones[:].offset,
                    ap=[list(ones[:].ap[0]), [0, LC * (H // 2)], [1, 64]],
                )
                base = v_sb[:, 0, :]
                for par, ooff in ((0, 64), (1, 128)):
                    dst = bass.AP(
                        tensor=base.tensor, offset=base.offset + ooff,
                        ap=[list(base.ap[0]), [VW, LC], [256, H // 2], [1, 64]],
                    )
                    nc.sync.dma_start(dst, ones_b)

            def scores_mc(q_sb, k_sb, c, p_tile, mc):
                """Scores (both heads of pair c, packed) + exp for chunk mc."""
                sc = scps.tile([128, 2, 512], F32, tag="sc")
                for par in range(2):
                    half = par * 64
                    nc.tensor.matmul(
                        sc[:, par, :],
                        k_sb[half:half + 64, c,
                             mc * 128:(mc + 1) * 128].opt(),
                        q_sb[half:half + 64, c, :].opt(),
                        start=True, stop=True,
                    )
                nc.scalar.activation(p_tile[:, mc, :, :], sc[:], EXP_FUNC)

            def pv_pair(v_sb, u_sb, c, p_tile):
                """PV chains + softmax normalization for head pair c."""
                u_ps_pair = []
                for par in range(2):
                    h = 2 * c + par
                    u_ps = mmps.tile([128, 512], F32, tag="ps")
                    for mc in range(LC):
                        nc.tensor.matmul(
                            u_ps[:],
                            v_sb[:, mc, h * 128:(h + 1) * 128].opt(),
                            p_tile[:, mc, par, :].opt(),
                            start=(mc == 0), stop=(mc == LC - 1),
                        )
                    u_ps_pair.append(u_ps)
                ups_a, ups_b = u_ps_pair
                # custom DVE ops (recip) only work at partition base 0,
                # so S_A is first cross-copied down to base 0; the
                # standard TT mul handles the in1 base crossing for B.
                tmp = rpool.tile([64, 512], F32, tag="rtmp")
                nc.vector.tensor_copy(tmp[0:64, :], ups_a[64:128, :])
                r_a = rpool.tile([64, 512], F32, tag="ra")
                nc.vector.reciprocal_approx_fast(r_a[0:64, :], tmp[0:64, :])
                r_b = rpool.tile([64, 512], F32, tag="rb")
                nc.vector.reciprocal_approx_fast(r_b[0:64, :], ups_b[0:64, :])
                nc.vector.tensor_mul(
                    u_sb[0:64, c, :], ups_a[0:64, :], r_a[0:64, :])
                nc.vector.tensor_mul(
                    u_sb[64:128, c, :], ups_b[64:128, :], r_b[0:64, :])
                if has_bv:
                    nc.vector.tensor_scalar_add(
                        u_sb[:, c, :], u_sb[:, c, :], bv_sb[:, c:c + 1])

            def wo_unit(pblk, pu_sb, unit):
                """One (lc, eh) output-projection unit of block pblk."""
                lc, eh = unit // 2, unit % 2
                ps = mmps.tile([128, 512], F32, tag="ps")
                for dc in range(KC):
                    nc.tensor.matmul(
                        ps[:],
                        pu_sb[:, dc, lc * 128:(lc + 1) * 128].opt(),
                        wo_sb[:, dc, eh * 512:(eh + 1) * 512].opt(),
                        start=(dc == 0),
                        stop=(dc == KC - 1),
                    )
                y_sb = ypool.tile([128, 512], BF16, tag="y_sb")
                nc.vector.tensor_copy(y_sb[:], ps[:])
                nc.sync.dma_start(
                    y[pblk, lc * 128:(lc + 1) * 128,
                      eh * 512:(eh + 1) * 512],
                    y_sb[:],
                )

            # ---- main schedule ---------------------------------------
            q_tiles = [None] * NBLK
            k_tiles = [None] * NBLK
            prev = None  # (blk, u_sb) with Wo still pending

            for blk in range(NBLK):
                zt_sb = zt_tiles[blk]

                # P1: block 0 computes its own Q/K; later blocks get them
                # as pairs-phase fillers of the previous block.
                if q_tiles[blk] is None:
                    q_sb = qkpool.tile([128, KC, L], BF16, tag="qk")
                    for dc in range(KC):
                        proj_chain(wq_sb, zt_sb, q_sb, dc, bq_sb)
                    q_tiles[blk] = q_sb
                if k_tiles[blk] is None:
                    k_sb = qkpool.tile([128, KC, L], BF16, tag="qk")
                    for dc in range(KC):
                        proj_chain(wk_sb, zt_sb, k_sb, dc, bk_sb)
                    k_tiles[blk] = k_sb
                q_sb, k_sb = q_tiles[blk], k_tiles[blk]
                v_sb = vpool.tile([128, LC, VW], BF16)
                v_ones(v_sb)
                # last block: nh=1 chains (only needed from PV pair 4 on)
                # move into the scalar-gated pairs steps as fillers
                last = blk == NBLK - 1
                for lc in range(LC):
                    for nh in range(1 if last else 2):
                        v_chain(blk, v_sb, lc, nh)

                # pairs-phase filler chains: prev block's Wo units, next
                # block's Q/K projection chains.
                fillers = []
                if prev is not None:
                    pblk, pu_sb = prev

                    def wo_filler(unit, pblk=pblk, pu_sb=pu_sb):
                        wo_unit(pblk, pu_sb, unit)

                    fillers.append(wo_filler)
                if last:
                    def v_filler(c, v_sb=v_sb, blk=blk):
                        if c < LC:
                            v_chain(blk, v_sb, c, 1)

                    fillers.append(v_filler)
                if blk + 1 < NBLK:
                    qn = qkpool.tile([128, KC, L], BF16, tag="qk", name="qn")
                    kn = qkpool.tile([128, KC, L], BF16, tag="qk", name="kn")
                    q_tiles[blk + 1] = qn
                    k_tiles[blk + 1] = kn
                    ztn = zt_tiles[blk + 1]

                    def q_filler(unit, ztn=ztn, qn=qn):
                        proj_chain(wq_sb, ztn, qn, unit, bq_sb)

                    def k_filler(unit, ztn=ztn, kn=kn):
                        proj_chain(wk_sb, ztn, kn, unit, bk_sb)

                    fillers.append(q_filler)
                    fillers.append(k_filler)

                u_sb = upool.tile([128, KC, L], BF16)
                p_tiles = [None] * NP
                for c in range(NP):
                    p_tile = ppool.tile([128, LC, 2, 512], BF16, tag="p")
                    p_tiles[c] = p_tile
                    scores_mc(q_sb, k_sb, c, p_tile, 0)
                    scores_mc(q_sb, k_sb, c, p_tile, 1)
                    if fillers:
                        fillers[0](c)
                    if c > 0:
                        pv_pair(v_sb, u_sb, c - 1, p_tiles[c - 1])
                    scores_mc(q_sb, k_sb, c, p_tile, 2)
                    scores_mc(q_sb, k_sb, c, p_tile, 3)
                    for f in fillers[1:]:
                        f(c)
                    # prefetch zt two blocks ahead mid-pairs
                    if c == 2 and blk + 2 < NBLK:
                        load_zt(blk + 2)
                pv_pair(v_sb, u_sb, NP - 1, p_tiles[NP - 1])

                prev = (blk, u_sb)

            # tail: last block's output projection
            pblk, pu_sb = prev
            for unit in range(2 * LC):
                wo_unit(pblk, pu_sb, unit)

    nc.finalize()
    return nc


_NC_CACHE = {}


def _get_nc(flags):
    if flags not in _NC_CACHE:
        _NC_CACHE[flags] = _build_nc(*flags)
    return _NC_CACHE[flags]


def _prep(x, Wq, bq, Wk, bk, Wv, bv, Wo, bo, layer_bit):
    x = np.asarray(x, dtype=np.float32)
    C = N // CHUNK
    ids = np.arange(C)
    partner = ids ^ (1 << int(layer_bit))
    a_idx = ids[ids < partner]
    b_idx = partner[ids < partner]
    P = a_idx.shape[0]

    xr = x.reshape(B, C, CHUNK, D)
    blocks = np.concatenate([xr[:, a_idx], xr[:, b_idx]], axis=2)  # [B,P,L,D]
    blocks = np.ascontiguousarray(
        blocks.transpose(1, 0, 3, 2).reshape(P * B, D, L).astype(ml_dtypes.bfloat16)
    )  # z^T per block
    scale = np.float32(1.0 / np.sqrt(DH))

    def chunkify(vec):  # [D] -> [128, KC] chunk-major per-partition scalars
        return np.ascontiguousarray(
            np.asarray(vec, np.float32).reshape(KC, 128).T
        )

    bf = ml_dtypes.bfloat16
    base = {
        "wq": np.ascontiguousarray((np.asarray(Wq, np.float32) * scale).astype(bf)),
        "wk": np.ascontiguousarray(np.asarray(Wk, np.float32).astype(bf)),
        "wv": np.ascontiguousarray(np.asarray(Wv, np.float32).astype(bf)),
        "wo": np.ascontiguousarray(np.asarray(Wo, np.float32).astype(bf)),
        "ones": np.ones((128, 64), bf),
    }
    has_bq = bool(np.any(np.asarray(bq))) if bq is not None else False
    has_bk = bool(np.any(np.asarray(bk))) if bk is not None else False
    has_bv = bool(np.any(np.asarray(bv))) if bv is not None else False
    if has_bq:
        base["bq"] = chunkify(np.asarray(bq, np.float32) * scale)
    if has_bk:
        base["bk"] = chunkify(bk)
    if has_bv:
        base["bv"] = chunkify(bv)

    in_maps = []
    for core in range(NCORES):
        m = dict(base)
        m["zt"] = blocks[core * NBLK:(core + 1) * NBLK]
        in_maps.append(m)
    return in_maps, (has_bq, has_bk, has_bv), (a_idx, b_idx, P)


def _gather(results, idxs, bo):
    a_idx, b_idx, P = idxs
    yb = np.concatenate(
        [np.asarray(r["y"], dtype=np.float32) for r in results], axis=0
    )  # [P*B, L, D]
    yb = yb.reshape(P, B, 2, CHUNK, D)
    out = np.empty((B, N // CHUNK, CHUNK, D), np.float32)
    out[:, a_idx] = yb[:, :, 0].transpose(1, 0, 2, 3)
    out[:, b_idx] = yb[:, :, 1].transpose(1, 0, 2, 3)
    out = out.reshape(B, N, D)
    bo = np.asarray(bo, np.float32) if bo is not None else None
    if bo is not None and np.any(bo):
        out = out + bo
    return out


def _run(inputs, trace=False):
    in_maps, flags, idxs = _prep(
        inputs["x"], inputs["Wq"], inputs.get("bq"), inputs["Wk"],
        inputs.get("bk"), inputs["Wv"], inputs.get("bv"), inputs["Wo"],
        inputs.get("bo"), inputs["layer_bit"],
    )
    nc = _get_nc(flags)
    res = run_bass_kernel_spmd(nc, in_maps, list(range(NCORES)), trace=trace)
    out = _gather(res.results, idxs, inputs.get("bo"))
    return out, res


def kernel(**inputs):
    out, _ = _run(inputs, trace=False)
    return out


def kernel_traced(**inputs):
    out, res = _run(inputs, trace=True)
    return out, res
